# revision 4
# baseline (speedup 1.0000x reference)
"""Trainium2 Bass kernel for nn_CrossAttention (B=2, T=V=4096, 16 heads, d=64).

Math: the reference einsums contract the k/v group axis g, so
  weight = softmax((x@Wq) @ (adj @ sum_g Wk_g)^T / sqrt(64))
  out    = (weight @ (adj @ sum_g Wv_g)) @ Wo + bo
The group fold (sum over g of Wk/Wv columns) is done host-side on the
weights; all tensor-sized compute runs on device.

Sharding: 8 cores = (batch b, quarter of T). Each core takes t-rows
[tq*1024, (tq+1)*1024) of batch b, needs adj[b] (redundant across the 4
cores of the same b), and writes its own out slice. No collectives.

v3 structure (post-trace rework of v2; v2 was jointly PE+ACT bound with
ACT ~85% busy on 512 exp instructions and PE ~89% busy):
  - kT/qT live on 64 partitions (K=64 contraction, no zero padding);
    one S matmul per (v-block, head-pair): out [128, 2*512] with the
    head pair packed in the moving AP  -> half the S instruction count.
  - One PV matmul per (v-block, head-pair): out [65, 2*512], moving
    P2 [128, 2*512]. Row 64 of O2 = softmax sums (ones column in vt).
  - Softmax exp is split across engines: even v-blocks run exact Exp on
    ACT (bf16 out), odd v-blocks run a Schraudolph-style integer-affine
    exp on DVE: i16 = round(S*log2e*16 + (127-C)*128), bitcast to bf16
    (C=0.0435 centers the log-linear sawtooth; DVE converts round-to-
    nearest, measured). Mixed-engine softmax rel-err ~9e-3 (<2e-2 gate).
  - Interleave: C1 (q^T for t-half 1) emitted as background tasks during
    D(t-half 0); first half of the out-projection during D(t-half 1).
"""

import numpy as np
import ml_dtypes

import concourse.bass as bass
import concourse.tile as tile
from concourse import bacc, mybir
from concourse.masks import make_identity

F32 = mybir.dt.float32
BF16 = mybir.dt.bfloat16
I16 = mybir.dt.int16
NP_BF16 = ml_dtypes.bfloat16

B = 2
T = 4096
V = 4096
E = 1024
HID = 1024
NH = 16
DH = 64
G = 4
N_CORES = 8
T_CORE = (B * T) // N_CORES  # 1024
P = 128

T_TILE = 512
ROW_G = 512           # rows per build stripe
SCALE = 1.0 / 8.0
# DVE Schraudolph exp: i16 = round(S_raw * A_DVE + B_DVE), bitcast bf16.
# code = (log2(P) + 127 - C)*128 with P = exp(S_raw/8), C = 0.0435.
A_DVE = float(np.float32(1.4426950408889634 * 128.0 / 8.0))
B_DVE = float(np.float32((127.0 - 0.0435) * 128.0))


def build_nc():
    EB = E // P                # 8
    DB = HID // P              # 8
    NVB = V // P               # 32
    NTT = T_CORE // T_TILE     # 2
    GC = ROW_G // P            # 4 chunks per stripe
    NSTRIPE_V = V // ROW_G     # 8

    nc = bacc.Bacc("TRN2", target_bir_lowering=False, debug=False,
                   num_devices=N_CORES)

    x_sl = nc.declare_dram_parameter("x_sl", [T_CORE, E], BF16, isOutput=False)
    adj_b = nc.declare_dram_parameter("adj_b", [V, E], BF16, isOutput=False)
    Wq = nc.declare_dram_parameter("Wq", [E, HID], BF16, isOutput=False)
    bq = nc.declare_dram_parameter("bq", [HID], F32, isOutput=False)
    Wkv = nc.declare_dram_parameter("Wkv", [E, P], BF16, isOutput=False)
    bk = nc.declare_dram_parameter("bk", [DH], F32, isOutput=False)
    bv = nc.declare_dram_parameter("bv", [DH], F32, isOutput=False)
    Wo = nc.declare_dram_parameter("Wo", [HID, HID], BF16, isOutput=False)
    bo = nc.declare_dram_parameter("bo", [HID], F32, isOutput=False)
    out_sl = nc.declare_dram_parameter("out_sl", [T_CORE, HID], F32,
                                       isOutput=True)
    sums_dram = nc.dram_tensor("sums_scratch", [NH, T_CORE], F32)

    def bcast_ap(param, n_part, n_free):
        a = param[:] if not isinstance(param, bass.AP) else param
        return bass.AP(tensor=a.tensor, offset=a.offset,
                       ap=[[0, n_part]] + list(a.ap))

    from contextlib import ExitStack
    with tile.TileContext(nc, pool_alloc_mode="queue") as tc, ExitStack() as st:
        consts = st.enter_context(tc.tile_pool(name="consts", bufs=1))
        persist = st.enter_context(tc.tile_pool(name="persist", bufs=1))
        # SBUF work pools.
        bw = st.enter_context(tc.tile_pool(name="bw", bufs=2))
        cw = st.enter_context(tc.tile_pool(name="cw", bufs=2))
        w1 = st.enter_context(tc.tile_pool(name="w1", bufs=1))
        dw = st.enter_context(tc.tile_pool(name="dw", bufs=3))
        dn = st.enter_context(tc.tile_pool(name="dn", bufs=2))
        ew = st.enter_context(tc.tile_pool(name="ew", bufs=2))

        ident = consts.tile([P, P], BF16)
        make_identity(nc, ident[:])
        bq_sb = consts.tile([P, DB], F32)
        nc.sync.dma_start(bq_sb[:], bq.rearrange("(db dp) -> dp db", dp=P))
        bk_sb = consts.tile([DH, 1], F32)
        nc.sync.dma_start(bk_sb[:], bk.rearrange("(a one) -> a one", one=1))
        bv_sb = consts.tile([P, 1], F32)
        nc.sync.dma_start(bv_sb[DH:P, :],
                          bv.rearrange("(a one) -> a one", one=1))
        bob = consts.tile([P, HID], F32)
        nc.gpsimd.dma_start(bob[:], bcast_ap(bo, P, HID))

        kT = persist.tile([DH, V], BF16)
        vt = persist.tile([P, NVB, DH + 1], BF16)
        qT = persist.tile([DH, NTT, NH, T_TILE], BF16)
        attnT = persist.tile([P, DB, T_CORE], BF16)
        nc.gpsimd.memset(vt[:, :, DH:DH + 1], 1.0)

        # Weight tiles; DMAs issued interleaved with the adj stream below
        # so the first stripes aren't queued behind 4MB of weights.
        Wq_sb = w1.tile([P, EB, HID], BF16)
        Wo_sb = w1.tile([P, DB, HID], BF16)
        Wkv_sb = w1.tile([P, EB, P], BF16)

        def transpose_block(dst_ap, src_ap, tr_pool, use_act=False):
            """PE-transpose a [128,128] bf16 block src -> dst (SBUF)."""
            ptr = tr_pool.tile([P, P], BF16, tag="ptr", name="ptr")
            nc.tensor.transpose(ptr[:], src_ap, ident[:])
            if use_act:
                nc.scalar.copy(dst_ap, ptr[:])
            else:
                nc.vector.tensor_copy(dst_ap, ptr[:])

        # ---- Phase B: K^T and V~ from adj (8 stripes of 512 rows) ----
        def b_stripe_dma(sv):
            r0 = sv * ROW_G
            adj_in = bw.tile([P, GC, E], BF16, tag="row_in", name="adj_in")
            nc.sync.dma_start(
                adj_in[:],
                adj_b[r0:r0 + ROW_G, :].rearrange("(c p) e -> p c e", p=P))
            return adj_in

        def emit_b_stripe(sv, adj_in, mm_pool, tr_pool):
            r0 = sv * ROW_G
            aT = bw.tile([P, EB, ROW_G], BF16, tag="aT")
            for eb in range(EB):
                for cc in range(GC):
                    transpose_block(aT[:, eb, cc * P:(cc + 1) * P],
                                    adj_in[:, cc, eb * P:(eb + 1) * P],
                                    tr_pool, use_act=(eb + cc) % 2 == 0)
            pkv = mm_pool.tile([P, ROW_G], F32, tag="proj", name="pkv")
            for eb in range(EB):
                nc.tensor.matmul(pkv[:], Wkv_sb[:, eb, :], aT[:, eb, :],
                                 start=(eb == 0), stop=(eb == EB - 1))
            nc.scalar.activation(kT[:, r0:r0 + ROW_G], pkv[0:DH, :],
                                 mybir.ActivationFunctionType.Identity,
                                 bias=bk_sb[:])
            vtmp = bw.tile([P, ROW_G], BF16, tag="vtmp")
            nc.vector.tensor_scalar_add(vtmp[DH:P, :], pkv[DH:P, :],
                                        bv_sb[DH:P, :])
            for cc in range(GC):
                vb = (r0 + cc * P) // P
                pvt = tr_pool.tile([P, DH], BF16, tag="pvt", name="pvt")
                nc.tensor.transpose(pvt[:], vtmp[DH:P, cc * P:(cc + 1) * P],
                                    ident[DH:P, DH:P])
                nc.vector.tensor_copy(vt[:, vb, 0:DH], pvt[:])

        # ---- Phase C: q^T for one t-half (emitted inline for tt=0,
        # as background tasks for tt=1) ----
        def c_tasks(tt, mm_pool, tr_pool, use_act=False):
            ts0 = tt * T_TILE  # noqa: F841 (DMA slice below)
            xT = cw.tile([P, EB, T_TILE], BF16, tag="xT", bufs=1)
            # DMA the whole 512-row half immediately (it overlaps whatever
            # else is running; the dependent transposes are deferred).
            x_in = cw.tile([P, GC, E], BF16, tag="x_in", bufs=1)
            nc.sync.dma_start(
                x_in[:],
                x_sl[ts0:ts0 + T_TILE, :].rearrange("(c p) e -> p c e", p=P))
            tasks = []
            for eb in range(EB):
                for cc in range(GC):
                    def tr_task(eb=eb, cc=cc):
                        transpose_block(xT[:, eb, cc * P:(cc + 1) * P],
                                        x_in[:, cc, eb * P:(eb + 1) * P],
                                        tr_pool,
                                        use_act=(use_act and
                                                 (eb + cc) % 2 == 0))
                    tasks.append(tr_task)
            for db in range(DB):
                pq_box = {}
                def mm_task(db=db, eb=0, pq_box=pq_box):
                    pq_box["pq"] = mm_pool.tile([P, T_TILE], F32, tag="proj", name="pq")
                    nc.tensor.matmul(pq_box["pq"][:],
                                     Wq_sb[:, 0, db * P:(db + 1) * P],
                                     xT[:, 0, :], start=True, stop=False)
                tasks.append(mm_task)
                for eb in range(1, EB):
                    def mm_task2(db=db, eb=eb, pq_box=pq_box):
                        nc.tensor.matmul(pq_box["pq"][:],
                                         Wq_sb[:, eb, db * P:(db + 1) * P],
                                         xT[:, eb, :],
                                         start=False, stop=(eb == EB - 1))
                    tasks.append(mm_task2)
                def evac_task(db=db, pq_box=pq_box, tt=tt):
                    pq = pq_box["pq"]
                    if use_act:
                        nc.scalar.activation(
                            qT[:, tt, 2 * db, :],
                            pq[0:DH, :],
                            mybir.ActivationFunctionType.Identity,
                            bias=bq_sb[0:DH, db:db + 1])
                    else:
                        nc.vector.tensor_scalar_add(
                            qT[:, tt, 2 * db, :],
                            pq[0:DH, :], bq_sb[0:DH, db:db + 1])
                    qtmp = cw.tile([P, T_TILE], BF16, tag="qtmp", bufs=1)
                    nc.vector.tensor_scalar_add(
                        qtmp[DH:P, :], pq[DH:P, :], bq_sb[DH:P, db:db + 1])
                    nc.gpsimd.dma_start(
                        qT[:, tt, 2 * db + 1, :],
                        qtmp[DH:P, :])
                tasks.append(evac_task)
            return tasks

        # ---- Phase E: out-projection tasks (16 groups of 9) ----
        def e_tasks(tc_lo, tc_hi, mm_pool):
            tasks = []
            for tc_i in range(tc_lo, tc_hi):
                for eh in range(HID // T_TILE):
                    po_box = {}
                    def mm0(tc_i=tc_i, eh=eh, po_box=po_box):
                        po_box["po"] = mm_pool.tile([P, T_TILE], F32, tag="proj", name="po")
                        nc.tensor.matmul(
                            po_box["po"][:],
                            attnT[:, 0, tc_i * P:(tc_i + 1) * P],
                            Wo_sb[:, 0, eh * T_TILE:(eh + 1) * T_TILE],
                            start=True, stop=False)
                    tasks.append(mm0)
                    for kb in range(1, DB):
                        def mmk(tc_i=tc_i, eh=eh, kb=kb, po_box=po_box):
                            nc.tensor.matmul(
                                po_box["po"][:],
                                attnT[:, kb, tc_i * P:(tc_i + 1) * P],
                                Wo_sb[:, kb, eh * T_TILE:(eh + 1) * T_TILE],
                                start=False, stop=(kb == DB - 1))
                        tasks.append(mmk)
                    def evac(tc_i=tc_i, eh=eh, po_box=po_box):
                        ot = ew.tile([P, T_TILE], F32, tag="ot")
                        nc.vector.tensor_add(
                            ot[:], po_box["po"][:],
                            bob[:, eh * T_TILE:(eh + 1) * T_TILE])
                        nc.sync.dma_start(
                            out_sl[tc_i * P:(tc_i + 1) * P,
                                   eh * T_TILE:(eh + 1) * T_TILE], ot[:])
                    tasks.append(evac)
            return tasks

        # ---- Emit prolog: B stripes, then C0 inline ----
        with (
            tc.tile_pool(name="mmpB", bufs=2, space="PSUM") as mmpB,
            tc.tile_pool(name="trpB", bufs=3, space="PSUM") as trpB,
        ):
            adj0 = b_stripe_dma(0)
            adj1 = b_stripe_dma(1)
            nc.sync.dma_start(Wkv_sb[:],
                              Wkv.rearrange("(eb ep) d -> ep eb d", ep=P))
            c0 = c_tasks(0, mmpB, trpB, use_act=True)  # issues the x(tt0) DMA now
            emit_b_stripe(0, adj0, mmpB, trpB)
            nc.sync.dma_start(Wq_sb[:],
                              Wq.rearrange("(eb ep) d -> ep eb d", ep=P))
            emit_b_stripe(1, adj1, mmpB, trpB)
            for sv in range(2, NSTRIPE_V):
                adj_in = b_stripe_dma(sv)
                emit_b_stripe(sv, adj_in, mmpB, trpB)
            nc.sync.dma_start(Wo_sb[:],
                              Wo.rearrange("(kb kp) e -> kp kb e", kp=P))
            for t in c0:
                t()

        # ---- Phase D with background-task interleave ----
        bg = []

        def drain(n=1):
            for _ in range(n):
                if bg:
                    bg.pop(0)()

        pend = []
        pend_pvs = [0]
        dctx = ExitStack()
        sp2 = dctx.enter_context(tc.tile_pool(name="sp2", bufs=2, space="PSUM"))
        op2 = dctx.enter_context(tc.tile_pool(name="op2", bufs=1, space="PSUM"))
        mmp1 = dctx.enter_context(tc.tile_pool(name="mmp1", bufs=1, space="PSUM"))
        trp1 = dctx.enter_context(tc.tile_pool(name="trp1", bufs=1, space="PSUM"))

        for tt in range(NTT):
            ts0 = tt * T_TILE
            if tt == 0:
                bg.extend(c_tasks(1, mmp1, trp1))
            else:
                bg.extend(e_tasks(0, T_TILE // P, mmp1))
            for g in range(NH // 2):
                h0 = 2 * g
                O2 = op2.tile([DH + 1, 2, T_TILE], F32, tag="O2", name="O2")
                for vb in range(NVB):
                    S2 = sp2.tile([P, 2, T_TILE], F32, tag="S2")
                    for hi in range(2):
                        nc.tensor.matmul(S2[:, hi, :],
                                         kT[:, vb * P:(vb + 1) * P],
                                         qT[:, tt, h0 + hi, :],
                                         start=True, stop=True)
                    P2 = dw.tile([P, 2, T_TILE], BF16, tag="P2", bufs=5,
                                 name="P2")
                    if vb % 2 == 0:
                        nc.scalar.activation(P2[:], S2[:],
                                             mybir.ActivationFunctionType.Exp,
                                             scale=SCALE)
                    else:
                        nc.vector.tensor_scalar(
                            P2[:].bitcast(I16), S2[:], A_DVE, B_DVE,
                            op0=mybir.AluOpType.mult,
                            op1=mybir.AluOpType.add)
                    for hi in range(2):
                        def pv_op(O2=O2, vb2=vb, hi=hi, P2=P2):
                            nc.tensor.matmul(
                                O2[:, hi, :], vt[:, vb2, :], P2[:, hi, :],
                                start=(vb2 == 0), stop=(vb2 == NVB - 1),
                                skip_group_check=True)
                        pend.append(("pv", pv_op))
                        pend_pvs[0] += 1
                    if vb == NVB - 1:
                        def norm_op(O2=O2, g=g, h0=h0, ts0=ts0):
                            onorm = dn.tile([DH + 1, 2, T_TILE], F32,
                                            tag="onorm", bufs=1, name="onorm")
                            nc.scalar.copy(onorm[:], O2[:])
                            nc.gpsimd.dma_start(
                                sums_dram[h0:h0 + 2, ts0:ts0 + T_TILE],
                                onorm[DH:DH + 1, :, :])
                            sbc = dn.tile([DH, 2, T_TILE], F32,
                                          tag="sbc", bufs=1, name="sbc")
                            nc.gpsimd.dma_start(
                                sbc[:],
                                bcast_ap(sums_dram[h0:h0 + 2,
                                                   ts0:ts0 + T_TILE],
                                         DH, 2 * T_TILE))
                            nc.vector.reciprocal_approx_fast(sbc[:], sbc[:])
                            nc.vector.tensor_mul(
                                attnT[0:DH, g, ts0:ts0 + T_TILE],
                                onorm[0:DH, 0, :], sbc[:, 0, :])
                            nrm = dn.tile([DH, T_TILE], BF16,
                                          tag="nrm", bufs=1, name="nrm")
                            nc.vector.tensor_mul(nrm[:], onorm[0:DH, 1, :],
                                                 sbc[:, 1, :])
                            nc.gpsimd.dma_start(
                                attnT[DH:P, g, ts0:ts0 + T_TILE], nrm[:])
                        pend.append(("norm", norm_op))
                    # Emit up to 2 pending PV matmuls per iteration,
                    # keeping >=4 queued so a PV trails its exp by ~2
                    # v-blocks and the in-order PE queue never stalls.
                    npv = 0
                    while pend and npv < 2:
                        kind, op = pend[0]
                        if kind == "pv":
                            if pend_pvs[0] <= 4:
                                break
                            pend_pvs[0] -= 1
                            npv += 1
                        pend.pop(0)
                        op()
                    drain(1)
        # flush pending PV / normalize work, then leftover bg tasks
        while pend:
            pend.pop(0)[1]()
        while bg:
            bg.pop(0)()
        dctx.close()

        # ---- E epilog: second t-half of the out-projection ----
        with tc.tile_pool(name="mmpE", bufs=3, space="PSUM") as mmpE:
            for t in e_tasks(T_TILE // P, T_CORE // P, mmpE):
                t()

    nc.compile()
    return nc


_NC = None


def _get_nc():
    global _NC
    if _NC is None:
        _NC = build_nc()
    return _NC


def _make_in_maps(inputs):
    x = np.asarray(inputs["x"], np.float32)
    adj = np.asarray(inputs["adj"], np.float32)
    Wq_f = np.asarray(inputs["Wq"], np.float32).astype(NP_BF16)
    bq_f = np.ascontiguousarray(np.asarray(inputs["bq"], np.float32))
    Wk_f = np.asarray(inputs["Wk"], np.float32).reshape(E, G, DH).sum(axis=1)
    bk_f = np.ascontiguousarray(
        np.asarray(inputs["bk"], np.float32).reshape(G, DH).sum(axis=0))
    Wv_f = np.asarray(inputs["Wv"], np.float32).reshape(E, G, DH).sum(axis=1)
    bv_f = np.ascontiguousarray(
        np.asarray(inputs["bv"], np.float32).reshape(G, DH).sum(axis=0))
    Wkv_f = np.ascontiguousarray(
        np.concatenate([Wk_f, Wv_f], axis=1).astype(NP_BF16))
    Wo_f = np.asarray(inputs["Wo"], np.float32).astype(NP_BF16)
    bo_f = np.ascontiguousarray(np.asarray(inputs["bo"], np.float32))

    in_maps = []
    for c in range(N_CORES):
        b = c // (N_CORES // B)
        tq = c % (N_CORES // B)
        in_maps.append({
            "x_sl": np.ascontiguousarray(
                x[b, tq * T_CORE:(tq + 1) * T_CORE, :].astype(NP_BF16)),
            "adj_b": np.ascontiguousarray(adj[b].astype(NP_BF16)),
            "Wq": Wq_f, "bq": bq_f, "Wkv": Wkv_f, "bk": bk_f,
            "bv": bv_f, "Wo": Wo_f, "bo": bo_f,
        })
    return in_maps


def kernel(x, adj, Wq, bq, Wk, bk, Wv, bv, Wo, bo):
    inputs = {"x": x, "adj": adj, "Wq": Wq, "bq": bq, "Wk": Wk, "bk": bk,
              "Wv": Wv, "bv": bv, "Wo": Wo, "bo": bo}
    nc = _get_nc()
    in_maps = _make_in_maps(inputs)

    from concourse.bass_utils import run_bass_kernel_spmd
    res = run_bass_kernel_spmd(nc, in_maps, list(range(N_CORES)))

    out = np.empty((B, T, HID), np.float32)
    for c in range(N_CORES):
        b = c // (N_CORES // B)
        tq = c % (N_CORES // B)
        out[b, tq * T_CORE:(tq + 1) * T_CORE, :] = res.results[c]["out_sl"]
    return out


# revision 5
# speedup vs baseline: 1.3211x; 1.3211x over previous
"""Trainium2 Bass kernel for nn_CrossAttention (B=2, T=V=4096, 16 heads, d=64).

Math: the reference einsums contract the k/v group axis g, so
  weight = softmax((x@Wq) @ (adj @ sum_g Wk_g)^T / sqrt(64))
  out    = (weight @ (adj @ sum_g Wv_g)) @ Wo + bo
The group fold (sum over g of Wk/Wv columns) is done host-side on the
weights; all tensor-sized compute runs on device.

Sharding: 8 cores = (batch b, quarter of T). Each core takes t-rows
[tq*1024, (tq+1)*1024) of batch b, needs adj[b] (redundant across the 4
cores of the same b), and writes its own out slice. No collectives.

v3 structure (post-trace rework of v2; v2 was jointly PE+ACT bound with
ACT ~85% busy on 512 exp instructions and PE ~89% busy):
  - kT/qT live on 64 partitions (K=64 contraction, no zero padding);
    one S matmul per (v-block, head-pair): out [128, 2*512] with the
    head pair packed in the moving AP  -> half the S instruction count.
  - One PV matmul per (v-block, head-pair): out [65, 2*512], moving
    P2 [128, 2*512]. Row 64 of O2 = softmax sums (ones column in vt).
  - Softmax exp is split across engines: even v-blocks run exact Exp on
    ACT (bf16 out), odd v-blocks run a Schraudolph-style integer-affine
    exp on DVE: i16 = round(S*log2e*16 + (127-C)*128), bitcast to bf16
    (C=0.0435 centers the log-linear sawtooth; DVE converts round-to-
    nearest, measured). Mixed-engine softmax rel-err ~9e-3 (<2e-2 gate).
  - Interleave: C1 (q^T for t-half 1) emitted as background tasks during
    D(t-half 0); first half of the out-projection during D(t-half 1).
"""

import numpy as np
import ml_dtypes

import concourse.bass as bass
import concourse.tile as tile
from concourse import bacc, mybir
from concourse.masks import make_identity

F32 = mybir.dt.float32
BF16 = mybir.dt.bfloat16
I16 = mybir.dt.int16
NP_BF16 = ml_dtypes.bfloat16

B = 2
T = 4096
V = 4096
E = 1024
HID = 1024
NH = 16
DH = 64
G = 4
N_CORES = 8
T_CORE = (B * T) // N_CORES  # 1024
P = 128

T_TILE = 512
ROW_G = 512           # rows per build stripe
SCALE = 1.0 / 8.0
# DVE Schraudolph exp: i16 = round(S_raw * A_DVE + B_DVE), bitcast bf16.
# code = (log2(P) + 127 - C)*128 with P = exp(S_raw/8), C = 0.0435.
A_DVE = float(np.float32(1.4426950408889634 * 128.0 / 8.0))
B_DVE = float(np.float32((127.0 - 0.0435) * 128.0))


def build_nc():
    EB = E // P                # 8
    DB = HID // P              # 8
    NVB = V // P               # 32
    NTT = T_CORE // T_TILE     # 2
    GC = ROW_G // P            # 4 chunks per stripe
    NSTRIPE_V = V // ROW_G     # 8

    nc = bacc.Bacc("TRN2", target_bir_lowering=False, debug=False,
                   num_devices=N_CORES)

    x_sl = nc.declare_dram_parameter("x_sl", [T_CORE, E], BF16, isOutput=False)
    adj_b = nc.declare_dram_parameter("adj_b", [V, E], BF16, isOutput=False)
    Wq = nc.declare_dram_parameter("Wq", [E, HID], BF16, isOutput=False)
    bq = nc.declare_dram_parameter("bq", [HID], F32, isOutput=False)
    Wkv = nc.declare_dram_parameter("Wkv", [E, P], BF16, isOutput=False)
    bk = nc.declare_dram_parameter("bk", [DH], F32, isOutput=False)
    bv = nc.declare_dram_parameter("bv", [DH], F32, isOutput=False)
    Wo = nc.declare_dram_parameter("Wo", [HID, HID], BF16, isOutput=False)
    bo = nc.declare_dram_parameter("bo", [HID], F32, isOutput=False)
    out_sl = nc.declare_dram_parameter("out_sl", [T_CORE, HID], F32,
                                       isOutput=True)
    sums_dram = nc.dram_tensor("sums_scratch", [NH, T_CORE], F32)

    def bcast_ap(param, n_part, n_free):
        a = param[:] if not isinstance(param, bass.AP) else param
        return bass.AP(tensor=a.tensor, offset=a.offset,
                       ap=[[0, n_part]] + list(a.ap))

    from contextlib import ExitStack
    with tile.TileContext(nc, pool_alloc_mode="queue") as tc, ExitStack() as st:
        consts = st.enter_context(tc.tile_pool(name="consts", bufs=1))
        persist = st.enter_context(tc.tile_pool(name="persist", bufs=1))
        # SBUF work pools.
        bw = st.enter_context(tc.tile_pool(name="bw", bufs=2))
        cw = st.enter_context(tc.tile_pool(name="cw", bufs=2))
        w1 = st.enter_context(tc.tile_pool(name="w1", bufs=1))
        dw = st.enter_context(tc.tile_pool(name="dw", bufs=3))
        dn = st.enter_context(tc.tile_pool(name="dn", bufs=2))
        ew = st.enter_context(tc.tile_pool(name="ew", bufs=2))

        ident = consts.tile([P, P], BF16)
        make_identity(nc, ident[:])
        bq_sb = consts.tile([P, DB], F32)
        nc.sync.dma_start(bq_sb[:], bq.rearrange("(db dp) -> dp db", dp=P))
        bk_sb = consts.tile([DH, 1], F32)
        nc.sync.dma_start(bk_sb[:], bk.rearrange("(a one) -> a one", one=1))
        bv_sb = consts.tile([P, 1], F32)
        nc.sync.dma_start(bv_sb[DH:P, :],
                          bv.rearrange("(a one) -> a one", one=1))
        bob = consts.tile([P, HID], F32)
        nc.gpsimd.dma_start(bob[:], bcast_ap(bo, P, HID))

        kT = persist.tile([P, V], BF16)
        vt = persist.tile([P, NVB, DH + 1], BF16)
        qT = persist.tile([P, NTT, NH, T_TILE], BF16)
        attnT = persist.tile([P, DB, T_CORE], BF16)
        nc.gpsimd.memset(kT[DH:P, :], 0.0)
        nc.gpsimd.memset(qT[DH:P, :, :, :], 0.0)
        nc.gpsimd.memset(vt[:, :, DH:DH + 1], 1.0)

        # Weight tiles; DMAs issued interleaved with the adj stream below
        # so the first stripes aren't queued behind 4MB of weights.
        Wq_sb = w1.tile([P, EB, HID], BF16)
        Wo_sb = w1.tile([P, DB, HID], BF16)
        Wkv_sb = w1.tile([P, EB, P], BF16)

        def transpose_block(dst_ap, src_ap, tr_pool, use_act=False):
            """PE-transpose a [128,128] bf16 block src -> dst (SBUF)."""
            ptr = tr_pool.tile([P, P], BF16, tag="ptr", name="ptr")
            nc.tensor.transpose(ptr[:], src_ap, ident[:])
            if use_act:
                nc.scalar.copy(dst_ap, ptr[:])
            else:
                nc.vector.tensor_copy(dst_ap, ptr[:])

        # ---- Phase B: K^T and V~ from adj (8 stripes of 512 rows) ----
        def b_stripe_dma(sv):
            r0 = sv * ROW_G
            adj_in = bw.tile([P, GC, E], BF16, tag="row_in", name="adj_in")
            nc.sync.dma_start(
                adj_in[:],
                adj_b[r0:r0 + ROW_G, :].rearrange("(c p) e -> p c e", p=P))
            return adj_in

        def emit_b_stripe(sv, adj_in, mm_pool, tr_pool):
            r0 = sv * ROW_G
            aT = bw.tile([P, EB, ROW_G], BF16, tag="aT")
            for eb in range(EB):
                for cc in range(GC):
                    transpose_block(aT[:, eb, cc * P:(cc + 1) * P],
                                    adj_in[:, cc, eb * P:(eb + 1) * P],
                                    tr_pool, use_act=(eb + cc) % 2 == 0)
            pkv = mm_pool.tile([P, ROW_G], F32, tag="proj", name="pkv")
            for eb in range(EB):
                nc.tensor.matmul(pkv[:], Wkv_sb[:, eb, :], aT[:, eb, :],
                                 start=(eb == 0), stop=(eb == EB - 1))
            nc.scalar.activation(kT[0:DH, r0:r0 + ROW_G], pkv[0:DH, :],
                                 mybir.ActivationFunctionType.Identity,
                                 bias=bk_sb[:])
            vtmp = bw.tile([P, ROW_G], BF16, tag="vtmp")
            nc.vector.tensor_scalar_add(vtmp[DH:P, :], pkv[DH:P, :],
                                        bv_sb[DH:P, :])
            for cc in range(GC):
                vb = (r0 + cc * P) // P
                pvt = tr_pool.tile([P, DH], BF16, tag="pvt", name="pvt")
                nc.tensor.transpose(pvt[:], vtmp[DH:P, cc * P:(cc + 1) * P],
                                    ident[DH:P, DH:P])
                nc.vector.tensor_copy(vt[:, vb, 0:DH], pvt[:])

        # ---- Phase C: q^T for one t-half (emitted inline for tt=0,
        # as background tasks for tt=1) ----
        def c_tasks(tt, mm_pool, tr_pool, use_act=False):
            ts0 = tt * T_TILE  # noqa: F841 (DMA slice below)
            xT = cw.tile([P, EB, T_TILE], BF16, tag="xT", bufs=1)
            # DMA the whole 512-row half immediately (it overlaps whatever
            # else is running; the dependent transposes are deferred).
            x_in = cw.tile([P, GC, E], BF16, tag="x_in", bufs=1)
            nc.sync.dma_start(
                x_in[:],
                x_sl[ts0:ts0 + T_TILE, :].rearrange("(c p) e -> p c e", p=P))
            tasks = []
            for eb in range(EB):
                for cc in range(GC):
                    def tr_task(eb=eb, cc=cc):
                        transpose_block(xT[:, eb, cc * P:(cc + 1) * P],
                                        x_in[:, cc, eb * P:(eb + 1) * P],
                                        tr_pool,
                                        use_act=(use_act and
                                                 (eb + cc) % 2 == 0))
                    tasks.append(tr_task)
            for db in range(DB):
                pq_box = {}
                def mm_task(db=db, eb=0, pq_box=pq_box):
                    pq_box["pq"] = mm_pool.tile([P, T_TILE], F32, tag="proj", name="pq")
                    nc.tensor.matmul(pq_box["pq"][:],
                                     Wq_sb[:, 0, db * P:(db + 1) * P],
                                     xT[:, 0, :], start=True, stop=False)
                tasks.append(mm_task)
                for eb in range(1, EB):
                    def mm_task2(db=db, eb=eb, pq_box=pq_box):
                        nc.tensor.matmul(pq_box["pq"][:],
                                         Wq_sb[:, eb, db * P:(db + 1) * P],
                                         xT[:, eb, :],
                                         start=False, stop=(eb == EB - 1))
                    tasks.append(mm_task2)
                def evac_task(db=db, pq_box=pq_box, tt=tt):
                    pq = pq_box["pq"]
                    if use_act:
                        nc.scalar.activation(
                            qT[0:DH, tt, 2 * db, :],
                            pq[0:DH, :],
                            mybir.ActivationFunctionType.Identity,
                            bias=bq_sb[0:DH, db:db + 1])
                    else:
                        nc.vector.tensor_scalar_add(
                            qT[0:DH, tt, 2 * db, :],
                            pq[0:DH, :], bq_sb[0:DH, db:db + 1])
                    qtmp = cw.tile([P, T_TILE], BF16, tag="qtmp", bufs=1)
                    nc.vector.tensor_scalar_add(
                        qtmp[DH:P, :], pq[DH:P, :], bq_sb[DH:P, db:db + 1])
                    nc.gpsimd.dma_start(
                        qT[0:DH, tt, 2 * db + 1, :],
                        qtmp[DH:P, :])
                tasks.append(evac_task)
            return tasks

        # ---- Phase E: out-projection tasks (16 groups of 9) ----
        def e_tasks(tc_lo, tc_hi, mm_pool):
            tasks = []
            for tc_i in range(tc_lo, tc_hi):
                for eh in range(HID // T_TILE):
                    po_box = {}
                    def mm0(tc_i=tc_i, eh=eh, po_box=po_box):
                        po_box["po"] = mm_pool.tile([P, T_TILE], F32, tag="proj", name="po")
                        nc.tensor.matmul(
                            po_box["po"][:],
                            attnT[:, 0, tc_i * P:(tc_i + 1) * P],
                            Wo_sb[:, 0, eh * T_TILE:(eh + 1) * T_TILE],
                            start=True, stop=False)
                    tasks.append(mm0)
                    for kb in range(1, DB):
                        def mmk(tc_i=tc_i, eh=eh, kb=kb, po_box=po_box):
                            nc.tensor.matmul(
                                po_box["po"][:],
                                attnT[:, kb, tc_i * P:(tc_i + 1) * P],
                                Wo_sb[:, kb, eh * T_TILE:(eh + 1) * T_TILE],
                                start=False, stop=(kb == DB - 1))
                        tasks.append(mmk)
                    def evac(tc_i=tc_i, eh=eh, po_box=po_box):
                        ot = ew.tile([P, T_TILE], F32, tag="ot")
                        nc.vector.tensor_add(
                            ot[:], po_box["po"][:],
                            bob[:, eh * T_TILE:(eh + 1) * T_TILE])
                        nc.sync.dma_start(
                            out_sl[tc_i * P:(tc_i + 1) * P,
                                   eh * T_TILE:(eh + 1) * T_TILE], ot[:])
                    tasks.append(evac)
            return tasks

        # ---- Emit prolog: B stripes, then C0 inline ----
        with (
            tc.tile_pool(name="mmpB", bufs=2, space="PSUM") as mmpB,
            tc.tile_pool(name="trpB", bufs=3, space="PSUM") as trpB,
        ):
            adj0 = b_stripe_dma(0)
            adj1 = b_stripe_dma(1)
            nc.sync.dma_start(Wkv_sb[:],
                              Wkv.rearrange("(eb ep) d -> ep eb d", ep=P))
            c0 = c_tasks(0, mmpB, trpB, use_act=True)  # issues the x(tt0) DMA now
            emit_b_stripe(0, adj0, mmpB, trpB)
            nc.sync.dma_start(Wq_sb[:],
                              Wq.rearrange("(eb ep) d -> ep eb d", ep=P))
            emit_b_stripe(1, adj1, mmpB, trpB)
            for sv in range(2, NSTRIPE_V):
                adj_in = b_stripe_dma(sv)
                emit_b_stripe(sv, adj_in, mmpB, trpB)
            nc.sync.dma_start(Wo_sb[:],
                              Wo.rearrange("(kb kp) e -> kp kb e", kp=P))
            for t in c0:
                t()

        # ---- Phase D with background-task interleave ----
        bg = []

        def drain(n=1):
            for _ in range(n):
                if bg:
                    bg.pop(0)()

        pend = []
        pend_pvs = [0]
        dctx = ExitStack()
        sp2 = dctx.enter_context(tc.tile_pool(name="sp2", bufs=2, space="PSUM"))
        op2 = dctx.enter_context(tc.tile_pool(name="op2", bufs=1, space="PSUM"))
        mmp1 = dctx.enter_context(tc.tile_pool(name="mmp1", bufs=1, space="PSUM"))
        trp1 = dctx.enter_context(tc.tile_pool(name="trp1", bufs=1, space="PSUM"))

        for tt in range(NTT):
            ts0 = tt * T_TILE
            if tt == 0:
                bg.extend(c_tasks(1, mmp1, trp1))
            else:
                bg.extend(e_tasks(0, T_TILE // P, mmp1))
            for g in range(NH // 2):
                h0 = 2 * g
                O2 = op2.tile([DH + 1, 2, T_TILE], F32, tag="O2", name="O2")
                for vb in range(NVB):
                    S2 = sp2.tile([P, 2, T_TILE], F32, tag="S2")
                    for hi in range(2):
                        nc.tensor.matmul(S2[:, hi, :],
                                         kT[:, vb * P:(vb + 1) * P],
                                         qT[:, tt, h0 + hi, :],
                                         start=True, stop=True)
                    P2 = dw.tile([P, 2, T_TILE], BF16, tag="P2", bufs=5,
                                 name="P2")
                    if vb % 2 == 0:
                        nc.scalar.activation(P2[:], S2[:],
                                             mybir.ActivationFunctionType.Exp,
                                             scale=SCALE)
                    else:
                        nc.vector.tensor_scalar(
                            P2[:].bitcast(I16), S2[:], A_DVE, B_DVE,
                            op0=mybir.AluOpType.mult,
                            op1=mybir.AluOpType.add)
                    for hi in range(2):
                        def pv_op(O2=O2, vb2=vb, hi=hi, P2=P2):
                            nc.tensor.matmul(
                                O2[:, hi, :], vt[:, vb2, :], P2[:, hi, :],
                                start=(vb2 == 0), stop=(vb2 == NVB - 1),
                                skip_group_check=True)
                        pend.append(("pv", pv_op))
                        pend_pvs[0] += 1
                    if vb == NVB - 1:
                        def norm_op(O2=O2, g=g, h0=h0, ts0=ts0):
                            onorm = dn.tile([DH + 1, 2, T_TILE], F32,
                                            tag="onorm", bufs=1, name="onorm")
                            nc.scalar.copy(onorm[:], O2[:])
                            nc.gpsimd.dma_start(
                                sums_dram[h0:h0 + 2, ts0:ts0 + T_TILE],
                                onorm[DH:DH + 1, :, :])
                            sbc = dn.tile([DH, 2, T_TILE], F32,
                                          tag="sbc", bufs=1, name="sbc")
                            nc.gpsimd.dma_start(
                                sbc[:],
                                bcast_ap(sums_dram[h0:h0 + 2,
                                                   ts0:ts0 + T_TILE],
                                         DH, 2 * T_TILE))
                            nc.vector.reciprocal_approx_fast(sbc[:], sbc[:])
                            nc.vector.tensor_mul(
                                attnT[0:DH, g, ts0:ts0 + T_TILE],
                                onorm[0:DH, 0, :], sbc[:, 0, :])
                            nrm = dn.tile([DH, T_TILE], BF16,
                                          tag="nrm", bufs=1, name="nrm")
                            nc.vector.tensor_mul(nrm[:], onorm[0:DH, 1, :],
                                                 sbc[:, 1, :])
                            nc.gpsimd.dma_start(
                                attnT[DH:P, g, ts0:ts0 + T_TILE], nrm[:])
                        pend.append(("norm", norm_op))
                    # Emit up to 2 pending PV matmuls per iteration,
                    # keeping >=4 queued so a PV trails its exp by ~2
                    # v-blocks and the in-order PE queue never stalls.
                    npv = 0
                    while pend and npv < 2:
                        kind, op = pend[0]
                        if kind == "pv":
                            if pend_pvs[0] <= 4:
                                break
                            pend_pvs[0] -= 1
                            npv += 1
                        pend.pop(0)
                        op()
                    drain(1)
        # flush pending PV / normalize work, then leftover bg tasks
        while pend:
            pend.pop(0)[1]()
        while bg:
            bg.pop(0)()
        dctx.close()

        # ---- E epilog: second t-half of the out-projection ----
        with tc.tile_pool(name="mmpE", bufs=3, space="PSUM") as mmpE:
            for t in e_tasks(T_TILE // P, T_CORE // P, mmpE):
                t()

    nc.compile()
    return nc


_NC = None


def _get_nc():
    global _NC
    if _NC is None:
        _NC = build_nc()
    return _NC


def _make_in_maps(inputs):
    x = np.asarray(inputs["x"], np.float32)
    adj = np.asarray(inputs["adj"], np.float32)
    Wq_f = np.asarray(inputs["Wq"], np.float32).astype(NP_BF16)
    bq_f = np.ascontiguousarray(np.asarray(inputs["bq"], np.float32))
    Wk_f = np.asarray(inputs["Wk"], np.float32).reshape(E, G, DH).sum(axis=1)
    bk_f = np.ascontiguousarray(
        np.asarray(inputs["bk"], np.float32).reshape(G, DH).sum(axis=0))
    Wv_f = np.asarray(inputs["Wv"], np.float32).reshape(E, G, DH).sum(axis=1)
    bv_f = np.ascontiguousarray(
        np.asarray(inputs["bv"], np.float32).reshape(G, DH).sum(axis=0))
    Wkv_f = np.ascontiguousarray(
        np.concatenate([Wk_f, Wv_f], axis=1).astype(NP_BF16))
    Wo_f = np.asarray(inputs["Wo"], np.float32).astype(NP_BF16)
    bo_f = np.ascontiguousarray(np.asarray(inputs["bo"], np.float32))

    in_maps = []
    for c in range(N_CORES):
        b = c // (N_CORES // B)
        tq = c % (N_CORES // B)
        in_maps.append({
            "x_sl": np.ascontiguousarray(
                x[b, tq * T_CORE:(tq + 1) * T_CORE, :].astype(NP_BF16)),
            "adj_b": np.ascontiguousarray(adj[b].astype(NP_BF16)),
            "Wq": Wq_f, "bq": bq_f, "Wkv": Wkv_f, "bk": bk_f,
            "bv": bv_f, "Wo": Wo_f, "bo": bo_f,
        })
    return in_maps


def kernel(x, adj, Wq, bq, Wk, bk, Wv, bv, Wo, bo):
    inputs = {"x": x, "adj": adj, "Wq": Wq, "bq": bq, "Wk": Wk, "bk": bk,
              "Wv": Wv, "bv": bv, "Wo": Wo, "bo": bo}
    nc = _get_nc()
    in_maps = _make_in_maps(inputs)

    from concourse.bass_utils import run_bass_kernel_spmd
    res = run_bass_kernel_spmd(nc, in_maps, list(range(N_CORES)))

    out = np.empty((B, T, HID), np.float32)
    for c in range(N_CORES):
        b = c // (N_CORES // B)
        tq = c % (N_CORES // B)
        out[b, tq * T_CORE:(tq + 1) * T_CORE, :] = res.results[c]["out_sl"]
    return out


# revision 6
# speedup vs baseline: 1.4451x; 1.0939x over previous
"""Trainium2 Bass kernel for nn_CrossAttention (B=2, T=V=4096, 16 heads, d=64).

Math: the reference einsums contract the k/v group axis g, so
  weight = softmax((x@Wq) @ (adj @ sum_g Wk_g)^T / sqrt(64))
  out    = (weight @ (adj @ sum_g Wv_g)) @ Wo + bo
The group fold (sum over g of Wk/Wv columns) is done host-side on the
weights; all tensor-sized compute runs on device.

Sharding: 8 cores = (batch b, quarter of T). Each core takes t-rows
[tq*1024, (tq+1)*1024) of batch b, needs adj[b] (redundant across the 4
cores of the same b), and writes its own out slice. No collectives.

v3 structure (post-trace rework of v2; v2 was jointly PE+ACT bound with
ACT ~85% busy on 512 exp instructions and PE ~89% busy):
  - kT/qT live on 64 partitions (K=64 contraction, no zero padding);
    one S matmul per (v-block, head-pair): out [128, 2*512] with the
    head pair packed in the moving AP  -> half the S instruction count.
  - One PV matmul per (v-block, head-pair): out [65, 2*512], moving
    P2 [128, 2*512]. Row 64 of O2 = softmax sums (ones column in vt).
  - Softmax exp is split across engines: even v-blocks run exact Exp on
    ACT (bf16 out), odd v-blocks run a Schraudolph-style integer-affine
    exp on DVE: i16 = round(S*log2e*16 + (127-C)*128), bitcast to bf16
    (C=0.0435 centers the log-linear sawtooth; DVE converts round-to-
    nearest, measured). Mixed-engine softmax rel-err ~9e-3 (<2e-2 gate).
  - Interleave: C1 (q^T for t-half 1) emitted as background tasks during
    D(t-half 0); first half of the out-projection during D(t-half 1).
"""

import numpy as np
import ml_dtypes

import concourse.bass as bass
import concourse.tile as tile
from concourse import bacc, mybir
from concourse.masks import make_identity

F32 = mybir.dt.float32
BF16 = mybir.dt.bfloat16
I16 = mybir.dt.int16
NP_BF16 = ml_dtypes.bfloat16

B = 2
T = 4096
V = 4096
E = 1024
HID = 1024
NH = 16
DH = 64
G = 4
N_CORES = 8
T_CORE = (B * T) // N_CORES  # 1024
P = 128

T_TILE = 512
ROW_G = 512           # rows per build stripe
SCALE = 1.0 / 8.0
# DVE Schraudolph exp: i16 = round(S_raw * A_DVE + B_DVE), bitcast bf16.
# code = (log2(P) + 127 - C)*128 with P = exp(S_raw/8), C = 0.0435.
A_DVE = float(np.float32(1.4426950408889634 * 128.0 / 8.0))
B_DVE = float(np.float32((127.0 - 0.0435) * 128.0))


def build_nc():
    EB = E // P                # 8
    DB = HID // P              # 8
    NVB = V // P               # 32
    NTT = T_CORE // T_TILE     # 2
    GC = ROW_G // P            # 4 chunks per stripe
    NSTRIPE_V = V // ROW_G     # 8

    nc = bacc.Bacc("TRN2", target_bir_lowering=False, debug=False,
                   num_devices=N_CORES)

    x_sl = nc.declare_dram_parameter("x_sl", [T_CORE, E], BF16, isOutput=False)
    adj_b = nc.declare_dram_parameter("adj_b", [V, E], BF16, isOutput=False)
    Wq = nc.declare_dram_parameter("Wq", [E, HID], BF16, isOutput=False)
    bq = nc.declare_dram_parameter("bq", [HID], F32, isOutput=False)
    Wkv = nc.declare_dram_parameter("Wkv", [E, P], BF16, isOutput=False)
    bk = nc.declare_dram_parameter("bk", [DH], F32, isOutput=False)
    bv = nc.declare_dram_parameter("bv", [DH], F32, isOutput=False)
    Wo = nc.declare_dram_parameter("Wo", [HID, HID], BF16, isOutput=False)
    bo = nc.declare_dram_parameter("bo", [HID], F32, isOutput=False)
    out_sl = nc.declare_dram_parameter("out_sl", [T_CORE, HID], F32,
                                       isOutput=True)
    sums_dram = nc.dram_tensor("sums_scratch", [NH, T_CORE], F32)

    def bcast_ap(param, n_part, n_free):
        a = param[:] if not isinstance(param, bass.AP) else param
        return bass.AP(tensor=a.tensor, offset=a.offset,
                       ap=[[0, n_part]] + list(a.ap))

    from contextlib import ExitStack
    with tile.TileContext(nc, pool_alloc_mode="queue") as tc, ExitStack() as st:
        consts = st.enter_context(tc.tile_pool(name="consts", bufs=1))
        persist = st.enter_context(tc.tile_pool(name="persist", bufs=1))
        # SBUF work pools.
        bw = st.enter_context(tc.tile_pool(name="bw", bufs=2))
        cw = st.enter_context(tc.tile_pool(name="cw", bufs=2))
        w1 = st.enter_context(tc.tile_pool(name="w1", bufs=1))
        dw = st.enter_context(tc.tile_pool(name="dw", bufs=3))
        dn = st.enter_context(tc.tile_pool(name="dn", bufs=2))
        ew = st.enter_context(tc.tile_pool(name="ew", bufs=2))

        ident = consts.tile([P, P], BF16)
        make_identity(nc, ident[:])
        bq_sb = consts.tile([P, DB], F32)
        nc.sync.dma_start(bq_sb[:], bq.rearrange("(db dp) -> dp db", dp=P))
        bk_sb = consts.tile([DH, 1], F32)
        nc.sync.dma_start(bk_sb[:], bk.rearrange("(a one) -> a one", one=1))
        bv_sb = consts.tile([P, 1], F32)
        nc.sync.dma_start(bv_sb[DH:P, :],
                          bv.rearrange("(a one) -> a one", one=1))
        bob = consts.tile([P, HID], F32)
        nc.gpsimd.dma_start(bob[:], bcast_ap(bo, P, HID))

        kT = persist.tile([P, V], BF16)
        vt = persist.tile([P, NVB, DH + 1], BF16)
        qT = persist.tile([P, NTT, NH, T_TILE], BF16)
        attnT = persist.tile([P, DB, T_CORE], BF16)
        nc.gpsimd.memset(kT[DH:P, :], 0.0)
        nc.gpsimd.memset(qT[DH:P, :, :, :], 0.0)
        nc.gpsimd.memset(vt[:, :, DH:DH + 1], 1.0)

        # Weight tiles; DMAs issued interleaved with the adj stream below
        # so the first stripes aren't queued behind 4MB of weights.
        Wq_sb = w1.tile([P, EB, HID], BF16)
        Wo_sb = w1.tile([P, DB, HID], BF16)
        Wkv_sb = w1.tile([P, EB, P], BF16)

        def transpose_block(dst_ap, src_ap, tr_pool, use_act=False):
            """PE-transpose a [128,128] bf16 block src -> dst (SBUF)."""
            ptr = tr_pool.tile([P, P], BF16, tag="ptr", name="ptr")
            nc.tensor.transpose(ptr[:], src_ap, ident[:])
            if use_act:
                nc.scalar.copy(dst_ap, ptr[:])
            else:
                nc.vector.tensor_copy(dst_ap, ptr[:])

        # ---- Phase B: K^T and V~ from adj (8 stripes of 512 rows) ----
        def b_stripe_dma(sv):
            r0 = sv * ROW_G
            adj_in = bw.tile([P, GC, E], BF16, tag="row_in", name="adj_in")
            nc.sync.dma_start(
                adj_in[:],
                adj_b[r0:r0 + ROW_G, :].rearrange("(c p) e -> p c e", p=P))
            return adj_in

        def emit_b_stripe(sv, adj_in, mm_pool, tr_pool):
            r0 = sv * ROW_G
            aT = bw.tile([P, EB, ROW_G], BF16, tag="aT")
            for eb in range(EB):
                for cc in range(GC):
                    transpose_block(aT[:, eb, cc * P:(cc + 1) * P],
                                    adj_in[:, cc, eb * P:(eb + 1) * P],
                                    tr_pool, use_act=(eb + cc) % 2 == 0)
            pkv = mm_pool.tile([P, ROW_G], F32, tag="proj", name="pkv")
            for eb in range(EB):
                nc.tensor.matmul(pkv[:], Wkv_sb[:, eb, :], aT[:, eb, :],
                                 start=(eb == 0), stop=(eb == EB - 1))
            nc.scalar.activation(kT[0:DH, r0:r0 + ROW_G], pkv[0:DH, :],
                                 mybir.ActivationFunctionType.Identity,
                                 bias=bk_sb[:])
            vtmp = bw.tile([P, ROW_G], BF16, tag="vtmp")
            nc.vector.tensor_scalar_add(vtmp[DH:P, :], pkv[DH:P, :],
                                        bv_sb[DH:P, :])
            for cc in range(GC):
                vb = (r0 + cc * P) // P
                pvt = tr_pool.tile([P, DH], BF16, tag="pvt", name="pvt")
                nc.tensor.transpose(pvt[:], vtmp[DH:P, cc * P:(cc + 1) * P],
                                    ident[DH:P, DH:P])
                nc.vector.tensor_copy(vt[:, vb, 0:DH], pvt[:])

        # ---- Phase C: q^T for one t-half (emitted inline for tt=0,
        # as background tasks for tt=1) ----
        def c_tasks(tt, mm_pool, tr_pool, use_act=False):
            ts0 = tt * T_TILE  # noqa: F841 (DMA slice below)
            xT = cw.tile([P, EB, T_TILE], BF16, tag="xT", bufs=1)
            # DMA the whole 512-row half immediately (it overlaps whatever
            # else is running; the dependent transposes are deferred).
            x_in = cw.tile([P, GC, E], BF16, tag="x_in", bufs=1)
            nc.sync.dma_start(
                x_in[:],
                x_sl[ts0:ts0 + T_TILE, :].rearrange("(c p) e -> p c e", p=P))
            tasks = []
            for eb in range(EB):
                for cc in range(GC):
                    def tr_task(eb=eb, cc=cc):
                        transpose_block(xT[:, eb, cc * P:(cc + 1) * P],
                                        x_in[:, cc, eb * P:(eb + 1) * P],
                                        tr_pool,
                                        use_act=(use_act and
                                                 (eb + cc) % 2 == 0))
                    tasks.append(tr_task)
            for db in range(DB):
                pq_box = {}
                def mm_task(db=db, eb=0, pq_box=pq_box):
                    pq_box["pq"] = mm_pool.tile([P, T_TILE], F32, tag="proj", name="pq")
                    nc.tensor.matmul(pq_box["pq"][:],
                                     Wq_sb[:, 0, db * P:(db + 1) * P],
                                     xT[:, 0, :], start=True, stop=False)
                tasks.append(mm_task)
                for eb in range(1, EB):
                    def mm_task2(db=db, eb=eb, pq_box=pq_box):
                        nc.tensor.matmul(pq_box["pq"][:],
                                         Wq_sb[:, eb, db * P:(db + 1) * P],
                                         xT[:, eb, :],
                                         start=False, stop=(eb == EB - 1))
                    tasks.append(mm_task2)
                def evac_task(db=db, pq_box=pq_box, tt=tt):
                    pq = pq_box["pq"]
                    if use_act:
                        nc.scalar.activation(
                            qT[0:DH, tt, 2 * db, :],
                            pq[0:DH, :],
                            mybir.ActivationFunctionType.Identity,
                            bias=bq_sb[0:DH, db:db + 1])
                    else:
                        nc.vector.tensor_scalar_add(
                            qT[0:DH, tt, 2 * db, :],
                            pq[0:DH, :], bq_sb[0:DH, db:db + 1])
                    qtmp = cw.tile([P, T_TILE], BF16, tag="qtmp", bufs=1)
                    nc.vector.tensor_scalar_add(
                        qtmp[DH:P, :], pq[DH:P, :], bq_sb[DH:P, db:db + 1])
                    nc.gpsimd.dma_start(
                        qT[0:DH, tt, 2 * db + 1, :],
                        qtmp[DH:P, :])
                tasks.append(evac_task)
            return tasks

        # ---- Phase E: out-projection tasks (16 groups of 9) ----
        def e_tasks(tc_lo, tc_hi, mm_pool):
            tasks = []
            for tc_i in range(tc_lo, tc_hi):
                for eh in range(HID // T_TILE):
                    po_box = {}
                    def mm0(tc_i=tc_i, eh=eh, po_box=po_box):
                        po_box["po"] = mm_pool.tile([P, T_TILE], F32, tag="proj", name="po")
                        nc.tensor.matmul(
                            po_box["po"][:],
                            attnT[:, 0, tc_i * P:(tc_i + 1) * P],
                            Wo_sb[:, 0, eh * T_TILE:(eh + 1) * T_TILE],
                            start=True, stop=False)
                    tasks.append(mm0)
                    for kb in range(1, DB):
                        def mmk(tc_i=tc_i, eh=eh, kb=kb, po_box=po_box):
                            nc.tensor.matmul(
                                po_box["po"][:],
                                attnT[:, kb, tc_i * P:(tc_i + 1) * P],
                                Wo_sb[:, kb, eh * T_TILE:(eh + 1) * T_TILE],
                                start=False, stop=(kb == DB - 1))
                        tasks.append(mmk)
                    def evac(tc_i=tc_i, eh=eh, po_box=po_box):
                        ot = ew.tile([P, T_TILE], F32, tag="ot")
                        nc.vector.tensor_add(
                            ot[:], po_box["po"][:],
                            bob[:, eh * T_TILE:(eh + 1) * T_TILE])
                        nc.sync.dma_start(
                            out_sl[tc_i * P:(tc_i + 1) * P,
                                   eh * T_TILE:(eh + 1) * T_TILE], ot[:])
                    tasks.append(evac)
            return tasks

        # ---- Emit prolog: B stripes, then C0 inline ----
        with (
            tc.tile_pool(name="mmpB", bufs=2, space="PSUM") as mmpB,
            tc.tile_pool(name="trpB", bufs=3, space="PSUM") as trpB,
        ):
            adj0 = b_stripe_dma(0)
            adj1 = b_stripe_dma(1)
            nc.sync.dma_start(Wkv_sb[:],
                              Wkv.rearrange("(eb ep) d -> ep eb d", ep=P))
            c0 = c_tasks(0, mmpB, trpB, use_act=True)  # issues the x(tt0) DMA now
            emit_b_stripe(0, adj0, mmpB, trpB)
            nc.sync.dma_start(Wq_sb[:],
                              Wq.rearrange("(eb ep) d -> ep eb d", ep=P))
            emit_b_stripe(1, adj1, mmpB, trpB)
            for sv in range(2, NSTRIPE_V):
                adj_in = b_stripe_dma(sv)
                emit_b_stripe(sv, adj_in, mmpB, trpB)
            nc.sync.dma_start(Wo_sb[:],
                              Wo.rearrange("(kb kp) e -> kp kb e", kp=P))
            for t in c0:
                t()

        # ---- Phase D with background-task interleave ----
        bg = []

        def drain(n=1):
            for _ in range(n):
                if bg:
                    bg.pop(0)()

        pend = []
        pend_pvs = [0]
        dctx = ExitStack()
        sp2 = dctx.enter_context(tc.tile_pool(name="sp2", bufs=4, space="PSUM"))
        op2 = dctx.enter_context(tc.tile_pool(name="op2", bufs=1, space="PSUM"))
        mmp1 = dctx.enter_context(tc.tile_pool(name="mmp1", bufs=1, space="PSUM"))
        trp1 = dctx.enter_context(tc.tile_pool(name="trp1", bufs=1, space="PSUM"))

        for tt in range(NTT):
            ts0 = tt * T_TILE
            if tt == 0:
                bg.extend(c_tasks(1, mmp1, trp1, use_act=True))
            else:
                bg.extend(e_tasks(0, T_TILE // P, mmp1))
            for g in range(NH // 2):
                h0 = 2 * g
                O2 = op2.tile([DH + 1, 2, T_TILE], F32, tag="O2", name="O2")
                for vb in range(NVB):
                    for hi in range(2):
                        S2 = sp2.tile([P, T_TILE], F32, tag="S2")
                        nc.tensor.matmul(S2[:],
                                         kT[:, vb * P:(vb + 1) * P],
                                         qT[:, tt, h0 + hi, :],
                                         start=True, stop=True)
                        P2 = dw.tile([P, T_TILE], BF16, tag="P2", bufs=10,
                                     name="P2")
                        if (2 * vb + hi) % 4 == 1:
                            nc.vector.tensor_scalar(
                                P2[:].bitcast(I16), S2[:], A_DVE, B_DVE,
                                op0=mybir.AluOpType.mult,
                                op1=mybir.AluOpType.add)
                        else:
                            nc.scalar.activation(
                                P2[:], S2[:],
                                mybir.ActivationFunctionType.Exp,
                                scale=SCALE)
                        def pv_op(O2=O2, vb2=vb, hi=hi, P2=P2):
                            nc.tensor.matmul(
                                O2[:, hi, :], vt[:, vb2, :], P2[:],
                                start=(vb2 == 0), stop=(vb2 == NVB - 1),
                                skip_group_check=True)
                        pend.append(("pv", pv_op))
                        pend_pvs[0] += 1
                    if vb == NVB - 1:
                        def norm_op(O2=O2, g=g, h0=h0, ts0=ts0):
                            onorm = dn.tile([DH + 1, 2, T_TILE], F32,
                                            tag="onorm", bufs=1, name="onorm")
                            nc.scalar.copy(onorm[:], O2[:])
                            nc.gpsimd.dma_start(
                                sums_dram[h0:h0 + 2, ts0:ts0 + T_TILE],
                                onorm[DH:DH + 1, :, :])
                            sbc = dn.tile([DH, 2, T_TILE], F32,
                                          tag="sbc", bufs=1, name="sbc")
                            nc.gpsimd.dma_start(
                                sbc[:],
                                bcast_ap(sums_dram[h0:h0 + 2,
                                                   ts0:ts0 + T_TILE],
                                         DH, 2 * T_TILE))
                            nc.vector.reciprocal_approx_fast(sbc[:], sbc[:])
                            nc.vector.tensor_mul(
                                attnT[0:DH, g, ts0:ts0 + T_TILE],
                                onorm[0:DH, 0, :], sbc[:, 0, :])
                            nrm = dn.tile([DH, T_TILE], BF16,
                                          tag="nrm", bufs=1, name="nrm")
                            nc.vector.tensor_mul(nrm[:], onorm[0:DH, 1, :],
                                                 sbc[:, 1, :])
                            nc.gpsimd.dma_start(
                                attnT[DH:P, g, ts0:ts0 + T_TILE], nrm[:])
                        pend.append(("norm", norm_op))
                    # Emit up to 2 pending PV matmuls per iteration,
                    # keeping >=4 queued so a PV trails its exp by ~2
                    # v-blocks and the in-order PE queue never stalls.
                    npv = 0
                    while pend and npv < 2:
                        kind, op = pend[0]
                        if kind == "pv":
                            if pend_pvs[0] <= 4:
                                break
                            pend_pvs[0] -= 1
                            npv += 1
                        pend.pop(0)
                        op()
                    drain(1)
        # flush pending PV / normalize work, then leftover bg tasks
        while pend:
            pend.pop(0)[1]()
        while bg:
            bg.pop(0)()
        dctx.close()

        # ---- E epilog: second t-half of the out-projection ----
        with tc.tile_pool(name="mmpE", bufs=3, space="PSUM") as mmpE:
            for t in e_tasks(T_TILE // P, T_CORE // P, mmpE):
                t()

    nc.compile()
    return nc


_NC = None


def _get_nc():
    global _NC
    if _NC is None:
        _NC = build_nc()
    return _NC


def _make_in_maps(inputs):
    x = np.asarray(inputs["x"], np.float32)
    adj = np.asarray(inputs["adj"], np.float32)
    Wq_f = np.asarray(inputs["Wq"], np.float32).astype(NP_BF16)
    bq_f = np.ascontiguousarray(np.asarray(inputs["bq"], np.float32))
    Wk_f = np.asarray(inputs["Wk"], np.float32).reshape(E, G, DH).sum(axis=1)
    bk_f = np.ascontiguousarray(
        np.asarray(inputs["bk"], np.float32).reshape(G, DH).sum(axis=0))
    Wv_f = np.asarray(inputs["Wv"], np.float32).reshape(E, G, DH).sum(axis=1)
    bv_f = np.ascontiguousarray(
        np.asarray(inputs["bv"], np.float32).reshape(G, DH).sum(axis=0))
    Wkv_f = np.ascontiguousarray(
        np.concatenate([Wk_f, Wv_f], axis=1).astype(NP_BF16))
    Wo_f = np.asarray(inputs["Wo"], np.float32).astype(NP_BF16)
    bo_f = np.ascontiguousarray(np.asarray(inputs["bo"], np.float32))

    in_maps = []
    for c in range(N_CORES):
        b = c // (N_CORES // B)
        tq = c % (N_CORES // B)
        in_maps.append({
            "x_sl": np.ascontiguousarray(
                x[b, tq * T_CORE:(tq + 1) * T_CORE, :].astype(NP_BF16)),
            "adj_b": np.ascontiguousarray(adj[b].astype(NP_BF16)),
            "Wq": Wq_f, "bq": bq_f, "Wkv": Wkv_f, "bk": bk_f,
            "bv": bv_f, "Wo": Wo_f, "bo": bo_f,
        })
    return in_maps


def kernel(x, adj, Wq, bq, Wk, bk, Wv, bv, Wo, bo):
    inputs = {"x": x, "adj": adj, "Wq": Wq, "bq": bq, "Wk": Wk, "bk": bk,
              "Wv": Wv, "bv": bv, "Wo": Wo, "bo": bo}
    nc = _get_nc()
    in_maps = _make_in_maps(inputs)

    from concourse.bass_utils import run_bass_kernel_spmd
    res = run_bass_kernel_spmd(nc, in_maps, list(range(N_CORES)))

    out = np.empty((B, T, HID), np.float32)
    for c in range(N_CORES):
        b = c // (N_CORES // B)
        tq = c % (N_CORES // B)
        out[b, tq * T_CORE:(tq + 1) * T_CORE, :] = res.results[c]["out_sl"]
    return out
